# revision 79
# baseline (speedup 1.0000x reference)
"""Multi-head attention (B=4, S=2048, d_model=1024, 16 heads x 64) on 8 trn2 cores.

Sharding: tensor-parallel over heads -- each core owns 2 heads (128 of the
1024 q/k/v dims and 128 rows of Wo's input dim). Each core computes a
partial output projection yT_c [1024, 8192] (fp16); the host sums the 8
partials and adds the combined bias (bo + bv @ Wo.T -- the V bias commutes
through softmax-weighted averaging since the weights sum to 1).

v24 design vs v2 baseline (506us -> ~413us, rel err 8.5e-3):
- the phase-A out-projection drain spreads 4 ot tiles per QKV chunk
  instead of one 16-tile burst (which serialized on its own yst copies
  through the 2-buffer psS rotation, ~2.5us per drain).
- DVE_KTS drops kt 15: its exp moves to Act so the DVE has an idle
  window across the A@V tail where the norm chain runs; also one fewer
  Schraudolph tile (rel err 8.9e-3 -> 8.5e-3).
- ALL q/k bias-adds and v copies run on Act via scalar.add/copy
  (Identity + per-partition bias AP needs no activation table, so it
  coexists with Exp without table reloads). DVE was the most-loaded
  engine (~65%) with its phase-A queue delaying qTb past combo 0's
  scores; emptying it balances Act/DVE at ~52/58% (-7us). The last
  batch's qh0 out-projection drains at the final combo's start instead
  of the exposed tail.
- scores use zero-padded per-head K stationaries (kz0 = [k_h0; 0],
  kz1 = [0; k_h1], each [128, s]): a 64-row stationary (64*128
  LDWEIGHTS) cannot be pulled ahead of in-flight matmuls and cost
  ~96ns on EVERY scores pair -- and poisoned the load pipelining of the
  A@V and transpose matmuls too. With full 128-row stationaries every
  matmul class runs at the 216ns warm ideal (-38us). The zero rows
  contract against the other head's q rows, leaving results unchanged.

Earlier layers of the design (v6/v13):
- v8 carries 64 ones-columns (cols 0:63; v values in 64:127): the A@V
  matmul emits softmax row sums on PSUM partitions 0..63 for free
  (output-partition count does not affect PE cycles), killing the fp32r
  rank-1 broadcast matmuls and the srow DVE round-trip. Sums sit at base
  partition 0 because the custom-DVE reciprocal mis-reads inputs whose
  base partition differs from the output's.
- softmax exp is split Act/DVE in STRICT alternation (odd kts on DVE):
  the Act engine alone (1113ns per [128,1024] exp tile) cannot keep
  pace with the PE's ~864ns of matmul work per kt step, which
  HAM-throttled the PE to 1.2 GHz (the single biggest loss in the
  baseline). The DVE tiles run a Schraudolph exp (uint16 saturating
  bits = A*psum + B, bitcast fp16; unsigned saturation clamps negative
  bits to +0.0, fixing the NaN the disabled int16 variant had).
  Alternation is stall-free only because the out-projection moved out
  of the combos (below); any engine serving two exp tiles within a
  2-kt window stalls the A@V chain.
- the out-projection for both q-halves drains during the NEXT batch's
  phase A (QKV), not inside a combo: an in-combo out_qh pushes
  exp (17.7us) + norm (2.4) + yst copies (8.8) past the two helper
  engines' combined per-combo budget (27.6us), which no assignment can
  fix; phase A has long PE stretches with near-idle helper engines.
- the 16 per-batch V transposes interleave into combo 0's kt loop with a
  dedicated 1-bank psT pool: no 4.4us transpose-only window (transposes
  don't count as PE-busy for HAM) and no psA WAR stall against phase-A
  copies.
- deferred norm/out-proj work drains at the start of the next batch's
  QKV phase, before the QKV psum copies enter the DVE FIFO.
- engine assignment tuned so neither Act nor DVE exceeds the PE's combo
  budget: v copies + half the yst copies on Act, q/k bias-adds + v8
  copies + norm + other yst half on DVE.
- DMA order lets the first QKV matmul start ~6us earlier (wq + half of
  x tile 0 first; wk/wv interleaved; wo last).
"""

import numpy as np

import concourse.bass as bass
import concourse.mybir as mybir
from concourse import bacc
from concourse.tile import TileContext
from concourse.masks import make_identity
from concourse.bass_utils import run_bass_kernel_spmd

N_HEAD = 16
D_HEAD = 64
D_MODEL = N_HEAD * D_HEAD  # 1024
B, S = 4, 2048
N_CORES = 8
HPC = N_HEAD // N_CORES  # heads per core = 2
HD = HPC * D_HEAD        # per-core head dims = 128

F32 = mybir.dt.float32
F16 = mybir.dt.float16
U16 = mybir.dt.uint16
AF = mybir.ActivationFunctionType
ALU = mybir.AluOpType
F16NP = np.float16

# ---- scale constants ----
SQ = 8.0          # q/k pre-scale: psum score = SQ^2 * 8 * s = 512 * s
C_SHIFT = 2.6     # att = exp(s - C_SHIFT); cancels in normalization
L2E = 1.4426950408889634
ACT_SCALE = 1.0 / 512.0
# DVE Schraudolph exp (uint16 saturating bits -> fp16): Act offload.
# uint16 saturation clamps negative bits to +0.0; the -60 offset centers
# the piecewise-linear mantissa error (sim: rel err 8.8e-3 at 8/16 tiles).
A16 = 1024.0 * L2E * ACT_SCALE
B16 = 15360.0 - 1024.0 * L2E * C_SHIFT - 60.0

_TRACE = False  # test harness can flip this for profiling

# kt indices whose exp runs on DVE (Schraudolph) instead of Act: the Act
# engine alone (1113ns per [128,1024] exp tile) cannot keep pace with the
# PE's ~864ns per kt step; splitting alternate tiles onto DVE removes the
# per-kt micro-stall that HAM-throttles the PE down to 1.2 GHz.
# odd kts except 15: kt 15's exp moves to Act so the DVE has a ~2.6us
# idle window across kts 14-17 (the A@V-only tail) -- exactly where the
# scheduler runs the 2.4us norm chain. With a full odd set the norm
# delayed exp(1) of the next combo and stalled scores via psS recycling
# (~1.2us per combo). Act absorbs the extra tile: its kt 14,15,0' run has
# 3.0us of PE cover (tail + scores-only steps) against 3.24us of work.
DVE_KTS = frozenset({1, 3, 5, 7, 9, 11})


def build_mha(b=B, s=S, dve_kts=DVE_KTS):
    """Build the per-core Bass program (SPMD; all cores run this)."""
    P = 128
    tok = b * s
    dmc = D_MODEL // P        # 8 contraction chunks
    n_tc = s // 512           # 512-token chunks per batch
    n_kt = s // P             # k chunks per batch
    n_qh = s // 1024          # q halves per batch

    nc = bacc.Bacc("TRN2", target_bir_lowering=False, debug=False)

    xT = nc.dram_tensor("xT", [P, dmc, tok], F16, kind="ExternalInput")
    wq = nc.dram_tensor("wq", [P, dmc, HD], F16, kind="ExternalInput")
    wk = nc.dram_tensor("wk", [P, dmc, HD], F16, kind="ExternalInput")
    wv = nc.dram_tensor("wv", [P, dmc, HD], F16, kind="ExternalInput")
    wo = nc.dram_tensor("wo", [HD, D_MODEL], F16, kind="ExternalInput")
    bq = nc.dram_tensor("bq", [HD, 1], F32, kind="ExternalInput")
    bk = nc.dram_tensor("bk", [HD, 1], F32, kind="ExternalInput")
    yT = nc.dram_tensor("yT", [D_MODEL, tok], F16, kind="ExternalOutput")

    with TileContext(nc) as tc:
        with (
            nc.allow_low_precision(reason="fp16 tiles feed the PE by design"),
            tc.tile_pool(name="const", bufs=1) as const,
            tc.tile_pool(name="xin", bufs=4) as xin,
            tc.tile_pool(name="stg", bufs=3) as stg,
            tc.tile_pool(name="att", bufs=8) as attp,
            tc.tile_pool(name="at16", bufs=2) as at16p,
            tc.tile_pool(name="out", bufs=4) as outp,
            tc.tile_pool(name="smal", bufs=4) as smal,
            tc.tile_pool(name="psA", bufs=1, space="PSUM") as psA,
            tc.tile_pool(name="psS", bufs=2, space="PSUM") as psS,
            tc.tile_pool(name="psO", bufs=1, space="PSUM") as psO,
            tc.tile_pool(name="psT", bufs=1, space="PSUM") as psT,
        ):
            # ---- constants (resident) ----
            wq_sb = const.tile([P, dmc, HD], F16)
            wk_sb = const.tile([P, dmc, HD], F16)
            wv_sb = const.tile([P, dmc, HD], F16)
            wo_sb = const.tile([HD, D_MODEL], F16)
            bq_sb = const.tile([HD, 1], F32)
            bk_sb = const.tile([HD, 1], F32)
            ident16 = const.tile([P, P], F16)
            make_identity(nc, ident16[:])
            ebias = const.tile([P, 1], F32)
            nc.vector.memset(ebias[:], -C_SHIFT)
            # v8[p, kt, h, 0:64]  = 1.0 (row-sum columns; written once --
            #   sums land on psum partitions 0:63 where the custom-DVE
            #   reciprocal can read them without a base-partition shift)
            # v8[p, kt, h, 64:128] = v[token kt*128+p, head h dim d]
            v8_bufs = [const.tile([P, n_kt, HPC, P], F16, name=f"v8_{i}")
                       for i in range(2)]
            for vb in v8_bufs:
                nc.vector.memset(vb[:, :, :, 0:D_HEAD], 1.0)
            # zero-padded per-head K tiles: kz0 = [k_h0 ; 0], kz1 = [0 ; k_h1]
            # as [128, s]. A 64-row scores stationary (64*128 LDWEIGHTS)
            # cannot be pulled ahead of in-flight matmuls (row-group
            # conflict) and costs ~96ns per kt pair; 128-row stationaries
            # (QKV, out-proj) load hidden. The zero rows contract to zero
            # against the other head's q rows, so results are unchanged.
            kz_bufs = [(const.tile([P, s], F16, name=f"kz0_{i}"),
                        const.tile([P, s], F16, name=f"kz1_{i}"))
                       for i in range(2)]
            for kz0_, kz1_ in kz_bufs:
                nc.vector.memset(kz0_[D_HEAD:P, :], 0.0)
                nc.vector.memset(kz1_[0:D_HEAD, :], 0.0)
            # DMA order tuned so the first QKV matmul can start ASAP: wq,
            # then the first half of x tile 0 (emitted in the batch loop),
            # then the remaining weights. wo is deferred further still.
            nc.sync.dma_start(wq_sb[:, 0:4, :], wq[:, 0:4, :])
            nc.sync.dma_start(bq_sb[:], bq[:, :])
            nc.sync.dma_start(bk_sb[:], bk[:, :])
            nc.sync.dma_start(wq_sb[:, 4:8, :], wq[:, 4:8, :])

            # HAM warm-up: ~40 dummy matmuls on the resident identity tile
            # while the first weight/x DMAs stream. The PE needs ~4us of
            # continuous matmul work to reach K=8/8 (2.4 GHz); without
            # this, the first ~15us of QKV run at the cold 1.2 GHz. The
            # dummies fill otherwise-idle DMA-wait time (no new deps).
            wps = psA.tile([P, 512], F32, tag="psA", name="warm")
            for _ in range(40):
                nc.tensor.matmul(wps[:, 0:P], ident16[:], ident16[:],
                                 start=True, stop=True)

            pending_out = []
            pending_norm = []
            xt_next = {}  # bi -> prefetched xt tile for that batch's t=0

            def norm_combo(pso, AT16, p0, q0):
                # rinv = 1/rowsums (sums on psum partitions 0..63 via the
                # ones columns; att@v values on partitions 64..127).
                # reciprocal_approx_fast (custom DVE op) requires its input
                # at base partition 0 -- hence the swapped v8 layout.
                rinv = smal.tile([D_HEAD, 1024], F32, tag="rinv")
                nc.vector.reciprocal_approx_fast(rinv[:], pso[0:D_HEAD, :])
                nc.vector.tensor_mul(
                    AT16[p0:p0 + D_HEAD, q0:q0 + 1024],
                    pso[D_HEAD:P, :], rinv[:])

            def out_qh(AT16, t0, q0, ots=None, yst_on_act=None):
                # output projection for one q-half: 8 ot x [128, 1024] tiles.
                # ots selects a subset (for spreading across kt steps);
                # yst_on_act picks the copy engine (defaults to ot parity).
                for ot in (range(D_MODEL // P) if ots is None else ots):
                    psy = psS.tile([P, 1024], F32, tag="pss")
                    for tj in range(2):
                        c0 = q0 + tj * 512
                        nc.tensor.matmul(
                            psy[:, tj * 512:(tj + 1) * 512],
                            wo_sb[:, ot * P:(ot + 1) * P],
                            AT16[:, c0:c0 + 512],
                            start=True,
                            stop=True,
                        )
                    yst = outp.tile([P, 1024], F16, tag="yst")
                    on_act = (ot % 2 == 1) if yst_on_act is None else yst_on_act
                    if on_act:
                        nc.scalar.copy(yst[:], psy[:])
                    else:
                        nc.vector.tensor_copy(yst[:], psy[:])
                    nc.sync.dma_start(
                        yT[ot * P:(ot + 1) * P, t0 + q0:t0 + q0 + 1024],
                        yst[:],
                    )

            for bi in range(b):
                t0 = bi * s
                v8 = v8_bufs[bi % 2]

                # ---- phase A: q/k/v projections ----
                # previous batch's deferred normalization runs on DVE now,
                # BEFORE this batch's psum copies enter the DVE FIFO; the
                # matching out-projection is emitted after chunk 0's matmuls
                # so the PE covers the DVE latency.
                if pending_norm:
                    norm_combo(*pending_norm.pop(0))
                qTb = stg.tile([HD, s], F16, tag="qTb")
                kz0, kz1 = kz_bufs[bi % 2]
                vTb = stg.tile([HD, s], F16, tag="vTb")
                out_items = []
                for t in range(n_tc):
                    c0 = t0 + t * 512
                    if t == 0 and bi in xt_next:
                        # tile was prefetched during the previous batch's
                        # combos, ahead of that batch's yT DMA backlog
                        xt = xt_next.pop(bi)
                    else:
                        xt = xin.tile([P, dmc, 512], F16, tag="xt")
                        if bi == 0 and t == 0:
                            nc.sync.dma_start(xt[:, 0:4, :],
                                              xT[:, 0:4, c0:c0 + 512])
                            nc.sync.dma_start(xt[:, 4:8, :],
                                              xT[:, 4:8, c0:c0 + 512])
                            nc.sync.dma_start(wk_sb[:], wk[:, :, :])
                            nc.sync.dma_start(wv_sb[:], wv[:, :, :])
                        else:
                            nc.sync.dma_start(xt[:], xT[:, :, c0:c0 + 512])
                            if bi == 0 and t == 1:
                                # wo (256KB) isn't needed until the first
                                # out-projection (~100us in); queueing it
                                # before xt(t1) delayed chunk 1's compute
                                # ~2us and HAM-re-throttled the early kernel
                                nc.sync.dma_start(wo_sb[:], wo[:, :])
                    for w_sb, b_sb, dst in (
                        (wq_sb, bq_sb, qTb),
                        (wk_sb, bk_sb, None),
                        (wv_sb, None, vTb),
                    ):
                        ps = psA.tile([P, 512], F32, tag="psA")
                        for c in range(dmc):
                            nc.tensor.matmul(
                                ps[:],
                                w_sb[:, c, :],
                                xt[:, c, :],
                                start=(c == 0),
                                stop=(c == dmc - 1),
                            )
                        cs = slice(t * 512, (t + 1) * 512)
                        if dst is None:
                            # k: split per head into the zero-padded tiles.
                            # On Act (Identity + per-partition bias AP):
                            # DVE is the loaded engine (~65%) and these
                            # adds sit right where its FIFO is busiest.
                            nc.scalar.add(
                                kz0[0:D_HEAD, cs], ps[0:D_HEAD, :],
                                b_sb[0:D_HEAD])
                            nc.scalar.add(
                                kz1[D_HEAD:P, cs], ps[D_HEAD:P, :],
                                b_sb[D_HEAD:P])
                        elif b_sb is None:
                            # v copy on Act: keeps the DVE FIFO short during
                            # phase A (DVE carries exp tiles + norm + yst)
                            nc.scalar.copy(dst[:, cs], ps[:])
                        else:
                            # q bias-add also on Act: empties DVE's phase-A
                            # queue so qTb is ready well before combo 0's
                            # scores (the S166 batch-boundary waits)
                            nc.scalar.add(dst[:, cs], ps[:], b_sb[:])
                    # drain the deferred q-half out-projections spread
                    # across the chunk loop, 4 ot tiles per chunk: a
                    # contiguous 16-tile burst serializes on its own yst
                    # copies through the 2-buffer psS rotation (~2.5us per
                    # drain); interleaving QKV matmuls between pairs gives
                    # each psy's WAR target 1.7us to clear. (In-combo
                    # spreading fails -- psS is contended by scores there --
                    # but phase A's psS pool is otherwise idle.)
                    if t == 0 and pending_out:
                        out_items = [(a8, tt, qq, ot)
                                     for (a8, tt, qq) in pending_out
                                     for ot in range(D_MODEL // P)]
                        pending_out.clear()
                    for a8, tt, qq, ot in out_items[4 * t:4 * (t + 1)]:
                        # NB: yst stays on the ot-parity engine split --
                        # forcing all spread-drain ysts onto DVE re-
                        # serializes psy release (measured 421.7 vs 414)
                        out_qh(a8, tt, qq, ots=[ot])

                # ---- attention combos ----
                if bi + 1 < b:
                    # prefetch next batch's first x tile now: its dma_start
                    # enters the sync queue BEFORE this batch's 64 yT output
                    # DMAs, removing the ~2us QKV wait at the batch boundary
                    xtn = xin.tile([P, dmc, 512], F16, tag="xt")
                    nc.sync.dma_start(
                        xtn[:], xT[:, :, (bi + 1) * s:(bi + 1) * s + 512])
                    xt_next[bi + 1] = xtn
                AT16 = at16p.tile([HD, s], F16, tag="AT16")

                first_combo = True
                for qh in range(n_qh):
                    q0 = qh * 1024
                    for h in range(HPC):
                        p0 = 64 * h
                        if (bi == b - 1 and qh == n_qh - 1 and h == HPC - 1
                                and pending_out):
                            # last batch has no next phase A to host the
                            # qh0 out-projection: drain it at this final
                            # combo's start (PE cover for its crunch, and
                            # ~3.5us less exposed tail)
                            out_qh(*pending_out.pop(0))
                        pso = psO.tile([P, 1024], F32, tag="pso")
                        atts = []
                        for kt in range(n_kt + 2):
                            if kt < n_kt:
                                att = attp.tile([P, 1024], F16, tag="att")
                                pss = psS.tile([P, 1024], F32, tag="pss")
                                kz = kz0 if h == 0 else kz1
                                for j in range(2):
                                    nc.tensor.matmul(
                                        pss[:, j * 512:(j + 1) * 512],
                                        kz[:, kt * P:(kt + 1) * P],
                                        qTb[0:P,
                                            q0 + j * 512:q0 + (j + 1) * 512],
                                        start=True,
                                        stop=True,
                                    )
                                if first_combo:
                                    # interleaved V transpose: chunk kt of
                                    # vTb -> v8 (used by A@V two kt later,
                                    # and by all later combos)
                                    vps_f = psT.tile([P, 512], F32, tag="psT")
                                    vps = vps_f[:, 0:P // 2].bitcast(F16)
                                    nc.tensor.transpose(
                                        vps, vTb[:, kt * P:(kt + 1) * P],
                                        ident16[:])
                                    nc.vector.tensor_copy(
                                        v8[:, kt, :, D_HEAD:P],
                                        vps.rearrange(
                                            "p (h d) -> p h d", d=D_HEAD),
                                    )
                                if kt in dve_kts:
                                    # Schraudolph exp: uint16 bits =
                                    # sat(A16*psum + B16); negatives clamp
                                    # to +0.0 via unsigned saturation
                                    nc.vector.tensor_scalar(
                                        att[:].bitcast(U16),
                                        pss[:],
                                        A16,
                                        B16,
                                        op0=ALU.mult,
                                        op1=ALU.add,
                                    )
                                else:
                                    nc.scalar.activation(
                                        att[:], pss[:], AF.Exp,
                                        bias=ebias[:], scale=ACT_SCALE,
                                    )
                                atts.append(att)
                            if kt == 1 and pending_norm:
                                norm_combo(*pending_norm.pop(0))

                            if kt >= 2:
                                ki = kt - 2
                                for j in range(2):
                                    nc.tensor.matmul(
                                        pso[:, j * 512:(j + 1) * 512],
                                        v8[:, ki, h, :],
                                        atts[ki][:, j * 512:(j + 1) * 512],
                                        start=(ki == 0),
                                        stop=(ki == n_kt - 1),
                                    )
                        first_combo = False
                        pending_norm.append((pso, AT16, p0, q0))
                        if h == HPC - 1:
                            pending_out.append((AT16, t0, q0))

            while pending_norm:
                norm_combo(*pending_norm.pop(0))
            while pending_out:
                out_qh(*pending_out.pop(0))
    nc.compile()
    return nc


def host_inputs(inputs, Wq, bq, Wk, bk, Wv, bv, Wo, bo):
    """Prepare per-core input maps (fp16 host-side conversions)."""
    b, s, dm = inputs.shape
    tok = b * s
    dmc = dm // 128
    x2 = np.asarray(inputs, np.float32).reshape(tok, dmc, 128)
    xT16 = np.ascontiguousarray(x2.transpose(2, 1, 0)).astype(F16NP)

    def wprep(W, sl, scale):  # [hd, dm] slice -> [128, dmc, hd] fp16
        w = (np.asarray(W, np.float32)[sl, :] * scale).T  # [dm, hd]
        return np.ascontiguousarray(
            w.reshape(dmc, 128, HD).transpose(1, 0, 2)).astype(F16NP)

    in_maps = []
    for c in range(N_CORES):
        sl = slice(c * HD, (c + 1) * HD)
        wo_c = np.ascontiguousarray(
            np.asarray(Wo, np.float32)[:, sl].T).astype(F16NP)  # [128, dm]
        in_maps.append({
            "xT": xT16,
            "wq": wprep(Wq, sl, SQ),
            "wk": wprep(Wk, sl, SQ),
            "wv": wprep(Wv, sl, 1.0),
            "wo": wo_c,
            "bq": np.ascontiguousarray(
                (np.asarray(bq, np.float32)[sl] * SQ).reshape(HD, 1)),
            "bk": np.ascontiguousarray(
                (np.asarray(bk, np.float32)[sl] * SQ).reshape(HD, 1)),
        })
    return in_maps


_NC_CACHE = {}


def _get_nc(b, s):
    key = (b, s)
    if key not in _NC_CACHE:
        _NC_CACHE[key] = build_mha(b=b, s=s)
    return _NC_CACHE[key]


def kernel(inputs, Wq, bq, Wk, bk, Wv, bv, Wo, bo):
    inputs = np.asarray(inputs, dtype=np.float32)
    b, s, dm = inputs.shape

    in_maps = host_inputs(inputs, Wq, bq, Wk, bk, Wv, bv, Wo, bo)
    nc = _get_nc(b, s)
    res = run_bass_kernel_spmd(
        nc, in_maps, core_ids=list(range(N_CORES)), trace=_TRACE
    )
    acc = res.results[0]["yT"].astype(np.float32)
    for c in range(1, N_CORES):
        acc += res.results[c]["yT"].astype(np.float32)
    bo_eff = (np.asarray(bo, np.float64)
              + np.asarray(bv, np.float64) @ np.asarray(Wo, np.float64).T)
    out = acc.T + bo_eff[None, :].astype(np.float32)
    if _TRACE:
        kernel.last_results = res
    return out.reshape(b, s, dm).astype(np.float32)
